# revision 12
# baseline (speedup 1.0000x reference)
"""Trainium2 Bass kernel for nn_Ensemble_51058571214927 (nms_detection).

Sharding: 12 (frame, class) shards over 8 cores x 2 slots (SPMD).
Device does: exact stable rank (score desc, index asc), scatter-sort,
blocked greedy NMS clustering, cluster-rank, cluster-table scatter,
merge MLP + softmax + weighted merge, scatter to global sorted rows.
Host does: class compaction (sharding), weight layout prep, shard-sum merge.
"""
import numpy as np

# ---------------------------------------------------------------- constants
B, N = 4, 4096
CAP = 1536            # per-class capacity (max observed 1429)
NT = CAP // 128       # 12 member tiles
NBLK = CAP // 128     # 12 scan blocks
MAXC = 16
DUMP = CAP * MAXC     # dump row in cluster table
CTROWS = 24704        # cluster table rows (>= DUMP+16, = 128*193; *7 = 1544*112)
COND = np.float32(0.2)
IOU = np.float32(0.3)
NEG = np.float32(-1e30)
CTIE = np.float32(1e12)
S16K = np.float32(16384.0)
RROUNDS = 4
PADORIG = np.float32(5000.0)

_cache = {}


def _build_program():
    import bir_fixups  # noqa: F401  (toolchain workarounds)
    import concourse.bass as bass
    import concourse.mybir as mybir
    from concourse.tile import TileContext
    from concourse.masks import make_identity

    f32, i32, i8 = mybir.dt.float32, mybir.dt.int32, mybir.dt.int8
    A = mybir.AluOpType
    AF = mybir.ActivationFunctionType
    AX = mybir.AxisListType.X

    nc = bass.Bass()

    # ------------------------------------------------------------- tensors
    cboxes = nc.dram_tensor("cboxes", [2, CAP, 7], f32, kind="ExternalInput")
    cscore = nc.dram_tensor("cscore", [2, CAP], f32, kind="ExternalInput")
    corig = nc.dram_tensor("corig", [2, CAP], f32, kind="ExternalInput")
    fscore = nc.dram_tensor("fscore", [2, N], f32, kind="ExternalInput")
    clab = nc.dram_tensor("clab", [2, 128], f32, kind="ExternalInput")
    w1bd = nc.dram_tensor("w1bd", [8, 112, 128], f32, kind="ExternalInput")
    w2bd = nc.dram_tensor("w2bd", [8, 128, 16], f32, kind="ExternalInput")
    onesbd_d = nc.dram_tensor("onesbd", [112, 16], f32, kind="ExternalInput")
    b1c_d = nc.dram_tensor("b1c", [128, 1], f32, kind="ExternalInput")
    b2c_d = nc.dram_tensor("b2c", [16, 1], f32, kind="ExternalInput")

    import os
    _dbg = "ExternalOutput" if os.environ.get("KDBG") else "Internal"
    sortedtab = [nc.dram_tensor(f"sortedtab{s}", [CAP + 128, 16], f32, kind=_dbg)
                 for s in range(2)]
    colbuf = nc.dram_tensor("colbuf", [2, 8, CAP], f32, kind=_dbg)   # key,x1,x2,y1,y2,a3,cid
    coljl = nc.dram_tensor("coljl", [2, NBLK, 128], f32)
    ctab = [nc.dram_tensor(f"ctab{s}", [CTROWS, 7], f32, kind=_dbg) for s in range(2)]

    oinfo = [nc.dram_tensor(f"oinfo{s}", [N, 9], f32, kind="ExternalOutput")
             for s in range(2)]
    dbgx = nc.dram_tensor("dbgx", [2, 128, 32], f32, kind=_dbg) if _dbg == "ExternalOutput" else None
    olead = [nc.dram_tensor(f"olead{s}", [N], f32, kind="ExternalOutput")
             for s in range(2)]

    def bcast(ap_1d, count):
        # DRAM 1-D region -> [128, count] partition broadcast AP
        return bass.AP(ap_1d.tensor, ap_1d.offset, [[0, 128], [1, count]])

    with TileContext(nc) as tc:
        with tc.tile_pool(name="pc", bufs=1) as pc, \
             tc.tile_pool(name="pb", bufs=1) as pb, \
             tc.tile_pool(name="pw", bufs=1) as pw, \
             tc.tile_pool(name="pk", bufs=1) as pk, \
             tc.tile_pool(name="ps", bufs=1, space="PSUM") as ps, \
             tc.tile_pool(name="ps1", bufs=1, space="PSUM") as ps1:

            # ---------------------------------------------------- constants
            ident = pc.tile([128, 128], f32, tag="ident")
            make_identity(nc, ident[:])
            IOC = pc.tile([128, 1], f32, tag="IOC")
            nc.gpsimd.iota(IOC[:], pattern=[[0, 1]], base=0, channel_multiplier=1,
                           allow_small_or_imprecise_dtypes=True)
            JROW = pc.tile([128, CAP], f32, tag="JROW")
            nc.gpsimd.iota(JROW[:], pattern=[[1, CAP]], base=0, channel_multiplier=0,
                           allow_small_or_imprecise_dtypes=True)
            FORIG = pc.tile([128, N], f32, tag="FORIG")
            nc.gpsimd.iota(FORIG[:], pattern=[[1, N]], base=0, channel_multiplier=0,
                           allow_small_or_imprecise_dtypes=True)
            LT128 = pc.tile([128, 128], f32, tag="LT128")
            nc.vector.tensor_scalar(out=LT128[:], in0=JROW[:, 0:128], scalar1=IOC[:, :1],
                                    scalar2=None, op0=A.is_lt)
            NEGBIG = pc.tile([128, 512], f32, tag="NEGBIG")
            nc.vector.memset(NEGBIG[:], float(NEG))
            NEG9 = pc.tile([128, 16], f32, tag="NEG9")
            nc.vector.memset(NEG9[:], -1e9)
            MONE = pc.tile([128, 1], f32, tag="MONE")
            nc.vector.memset(MONE[:], -1.0)
            DUMPC = pc.tile([128, 1], f32, tag="DUMPC")
            nc.vector.memset(DUMPC[:], float(DUMP))
            ZER16 = pc.tile([128, 16], f32, tag="ZER16")
            nc.vector.memset(ZER16[:], 0.0)
            ZBIG = pc.tile([128, 1408], f32, tag="ZBIG")
            nc.vector.memset(ZBIG[:], 0.0)
            W1T = [pc.tile([112, 128], f32, tag=f"W1T{i}", name=f"W1T{i}") for i in range(8)]
            for i in range(8):
                nc.sync.dma_start(out=W1T[i][:], in_=w1bd[i])
            W2T = [pc.tile([128, 16], f32, tag=f"W2T{i}", name=f"W2T{i}") for i in range(8)]
            for i in range(8):
                nc.sync.dma_start(out=W2T[i][:], in_=w2bd[i])
            ONESBD = pc.tile([112, 16], f32, tag="ONESBD")
            nc.sync.dma_start(out=ONESBD[:], in_=onesbd_d[:])
            B1C = pc.tile([128, 1], f32, tag="B1C")
            nc.sync.dma_start(out=B1C[:], in_=b1c_d[:])
            B2C = pc.tile([16, 1], f32, tag="B2C")
            nc.sync.dma_start(out=B2C[:], in_=b2c_d[:])

            bcreg = nc.gpsimd.alloc_register("bcreg")
            nc.gpsimd.reg_mov(bcreg, N - 1)

            # zero outputs/scratch with partition-contiguous APs (few descriptors)
            def zfill(handle, nelem):
                per = nelem // 128
                assert per * 128 == nelem and per <= 1408
                nc.gpsimd.dma_start(
                    out=bass.AP(handle[:].tensor, 0, [[per, 128], [1, per]]),
                    in_=ZBIG[:, 0:per])
            for s in range(2):
                zfill(oinfo[s], N * 9)
                zfill(olead[s], N)
                zfill(sortedtab[s], (CAP + 128) * 16)
                zfill(ctab[s], CTROWS * 7)

            # ================================================= per-slot body
            for s in range(2):
                # ---- A: load member-major, geometry, key
                BX = pw.tile([128, NT, 7], f32, tag="BX")
                nc.sync.dma_start(out=BX[:], in_=cboxes[s].rearrange("(t p) c -> p t c", p=128))
                SC = pw.tile([128, NT], f32, tag="SC")
                nc.sync.dma_start(out=SC[:], in_=cscore[s].rearrange("(t p) -> p t", p=128))
                OG = pw.tile([128, NT], f32, tag="OG")
                nc.sync.dma_start(out=OG[:], in_=corig[s].rearrange("(t p) -> p t", p=128))
                CLABc = pw.tile([128, 1], f32, tag="CLABc")
                nc.sync.dma_start(out=CLABc[:], in_=clab[s].rearrange("(p a) -> p a", a=1))

                v01 = pw.tile([128, NT], f32, tag="v01")
                nc.vector.tensor_scalar(out=v01[:], in0=SC[:], scalar1=float(COND),
                                        scalar2=None, op0=A.is_gt)
                v8 = pw.tile([128, NT], i8, tag="v8")
                nc.vector.tensor_copy(out=v8[:], in_=v01[:])
                KEY = pw.tile([128, NT], f32, tag="KEY")
                nc.vector.select(out=KEY[:], mask=v8[:], on_true=SC[:],
                                 on_false=NEGBIG[:, 0:NT])

                OFFc = pw.tile([128, 1], f32, tag="OFFc")
                nc.vector.tensor_scalar(out=OFFc[:], in0=CLABc[:], scalar1=1e4,
                                        scalar2=None, op0=A.mult)
                cx = pw.tile([128, NT], f32, tag="cx")
                nc.vector.tensor_scalar(out=cx[:], in0=BX[:, :, 0], scalar1=OFFc[:, :1],
                                        scalar2=None, op0=A.add)
                hx = pw.tile([128, NT], f32, tag="hx")
                nc.vector.tensor_scalar(out=hx[:], in0=BX[:, :, 3], scalar1=0.5,
                                        scalar2=None, op0=A.mult)
                hy = pw.tile([128, NT], f32, tag="hy")
                nc.vector.tensor_scalar(out=hy[:], in0=BX[:, :, 4], scalar1=0.5,
                                        scalar2=None, op0=A.mult)
                gx1 = pw.tile([128, NT], f32, tag="gx1")
                nc.vector.tensor_tensor(out=gx1[:], in0=cx[:], in1=hx[:], op=A.subtract)
                gx2 = pw.tile([128, NT], f32, tag="gx2")
                nc.vector.tensor_tensor(out=gx2[:], in0=cx[:], in1=hx[:], op=A.add)
                gy1 = pw.tile([128, NT], f32, tag="gy1")
                nc.vector.tensor_tensor(out=gy1[:], in0=BX[:, :, 1], in1=hy[:], op=A.subtract)
                gy2 = pw.tile([128, NT], f32, tag="gy2")
                nc.vector.tensor_tensor(out=gy2[:], in0=BX[:, :, 1], in1=hy[:], op=A.add)
                wd = pw.tile([128, NT], f32, tag="wd")
                nc.vector.tensor_tensor(out=wd[:], in0=gx2[:], in1=gx1[:], op=A.subtract)
                hg = pw.tile([128, NT], f32, tag="hg")
                nc.vector.tensor_tensor(out=hg[:], in0=gy2[:], in1=gy1[:], op=A.subtract)
                ar = pw.tile([128, NT], f32, tag="ar")
                nc.vector.tensor_tensor(out=ar[:], in0=wd[:], in1=hg[:], op=A.mult)
                a3 = pw.tile([128, NT], f32, tag="a3")
                nc.vector.tensor_scalar(out=a3[:], in0=ar[:], scalar1=float(IOU),
                                        scalar2=None, op0=A.mult)

                # ---- B: class rank (key desc, orig asc)
                nc.gpsimd.dma_start(out=colbuf[s, 0].rearrange("(t p) -> p t", p=128),
                                    in_=KEY[:])
                keyB = pb.tile([128, CAP], f32, tag="keyB")
                nc.gpsimd.dma_start(out=keyB[:], in_=bcast(colbuf[s, 0], CAP))
                origB = pb.tile([128, CAP], f32, tag="origB")
                nc.gpsimd.dma_start(out=origB[:], in_=bcast(corig[s], CAP))

                CR = pw.tile([128, NT], f32, tag="CR")
                junk = pk.tile([128, 512], f32, tag="junk")
                for t in range(NT):
                    acc = None
                    for c0 in range(0, CAP, 512):
                        t1 = pk.tile([128, 512], f32, tag="t1")
                        nc.vector.tensor_scalar(out=t1[:], in0=keyB[:, c0:c0 + 512],
                                                scalar1=KEY[:, t:t + 1], scalar2=float(CTIE),
                                                op0=A.subtract, op1=A.mult)
                        r1 = pk.tile([128, 512], f32, tag="r1")
                        nc.vector.tensor_scalar(out=r1[:], in0=origB[:, c0:c0 + 512],
                                                scalar1=OG[:, t:t + 1], scalar2=None,
                                                op0=A.subtract)
                        part = pk.tile([128, 1], f32, tag="part")
                        nc.vector.scalar_tensor_tensor(out=junk[:], in0=t1[:], scalar=0.0,
                                                       in1=r1[:], op0=A.add, op1=A.is_gt,
                                                       accum_out=part[:])
                        if acc is None:
                            acc = pk.tile([128, 1], f32, tag="acc")
                            nc.vector.tensor_copy(out=acc[:], in_=part[:])
                        else:
                            nc.vector.tensor_tensor(out=acc[:], in0=acc[:], in1=part[:], op=A.add)
                    nc.vector.tensor_copy(out=CR[:, t:t + 1], in_=acc[:])

                # ---- C: global rank g
                fkB = pb.tile([128, N], f32, tag="fkB")
                for c0 in range(0, N, 512):
                    fsc = pk.tile([128, 512], f32, tag="fsc")
                    nc.gpsimd.dma_start(out=fsc[:], in_=bcast(fscore[s, c0:c0 + 512], 512))
                    fv = pk.tile([128, 512], f32, tag="fv")
                    nc.vector.tensor_scalar(out=fv[:], in0=fsc[:],
                                            scalar1=float(COND), scalar2=None, op0=A.is_gt)
                    fv8 = pk.tile([128, 512], i8, tag="fv8")
                    nc.vector.tensor_copy(out=fv8[:], in_=fv[:])
                    nc.vector.select(out=fkB[:, c0:c0 + 512], mask=fv8[:],
                                     on_true=fsc[:], on_false=NEGBIG[:])
                G = pw.tile([128, NT], f32, tag="G")
                for t in range(NT):
                    acc = None
                    for c0 in range(0, N, 512):
                        t1 = pk.tile([128, 512], f32, tag="t1")
                        nc.vector.tensor_scalar(out=t1[:], in0=fkB[:, c0:c0 + 512],
                                                scalar1=KEY[:, t:t + 1], scalar2=float(CTIE),
                                                op0=A.subtract, op1=A.mult)
                        r1 = pk.tile([128, 512], f32, tag="r1")
                        nc.vector.tensor_scalar(out=r1[:], in0=FORIG[:, c0:c0 + 512],
                                                scalar1=OG[:, t:t + 1], scalar2=None,
                                                op0=A.subtract)
                        part = pk.tile([128, 1], f32, tag="part")
                        nc.vector.scalar_tensor_tensor(out=junk[:], in0=t1[:], scalar=0.0,
                                                       in1=r1[:], op0=A.add, op1=A.is_gt,
                                                       accum_out=part[:])
                        if acc is None:
                            acc = pk.tile([128, 1], f32, tag="acc")
                            nc.vector.tensor_copy(out=acc[:], in_=part[:])
                        else:
                            nc.vector.tensor_tensor(out=acc[:], in0=acc[:], in1=part[:], op=A.add)
                    nc.vector.tensor_copy(out=G[:, t:t + 1], in_=acc[:])

                # ---- D: stage + scatter to sorted order
                ST = pw.tile([128, NT, 16], f32, tag="STG")
                nc.vector.tensor_copy(out=ST[:, :, 0:7], in_=BX[:])
                nc.vector.tensor_copy(out=ST[:, :, 7], in_=KEY[:])
                nc.vector.tensor_copy(out=ST[:, :, 8], in_=gx1[:])
                nc.vector.tensor_copy(out=ST[:, :, 9], in_=gx2[:])
                nc.vector.tensor_copy(out=ST[:, :, 10], in_=gy1[:])
                nc.vector.tensor_copy(out=ST[:, :, 11], in_=gy2[:])
                nc.vector.tensor_copy(out=ST[:, :, 12], in_=a3[:])
                nc.vector.tensor_copy(out=ST[:, :, 13], in_=SC[:])
                nc.vector.tensor_copy(out=ST[:, :, 14], in_=G[:])
                nc.vector.tensor_copy(out=ST[:, :, 15], in_=CR[:])
                CRi = pw.tile([128, NT], i32, tag="CRi")
                nc.vector.tensor_copy(out=CRi[:], in_=CR[:])
                for t in range(NT):
                    nc.gpsimd.indirect_dma_start(
                        out=sortedtab[s][:], in_=ST[:, t, :],
                        out_offset=bass.IndirectOffsetOnAxis(ap=CRi[:, t:t + 1], axis=0),
                        in_offset=None)

                # ---- E: reload sorted + broadcast geometry rows
                SRT = pw.tile([128, NT, 16], f32, tag="SRT")
                nc.gpsimd.dma_start(
                    out=SRT[:], in_=sortedtab[s][0:CAP].rearrange("(t p) c -> p t c", p=128))
                for ci, col in ((1, 8), (2, 9), (3, 10), (4, 11), (5, 12)):
                    nc.gpsimd.dma_start(out=colbuf[s, ci].rearrange("(t p) -> p t", p=128),
                                        in_=SRT[:, :, col])
                x1B = pb.tile([128, CAP], f32, tag="x1B")
                nc.gpsimd.dma_start(out=x1B[:], in_=bcast(colbuf[s, 1], CAP))
                x2B = pb.tile([128, CAP], f32, tag="x2B")
                nc.gpsimd.dma_start(out=x2B[:], in_=bcast(colbuf[s, 2], CAP))
                y1B = pb.tile([128, CAP], f32, tag="y1B")
                nc.gpsimd.dma_start(out=y1B[:], in_=bcast(colbuf[s, 3], CAP))
                y2B = pb.tile([128, CAP], f32, tag="y2B")
                nc.gpsimd.dma_start(out=y2B[:], in_=bcast(colbuf[s, 4], CAP))
                a3B = pb.tile([128, CAP], f32, tag="a3B")
                nc.gpsimd.dma_start(out=a3B[:], in_=bcast(colbuf[s, 5], CAP))

                VS = pw.tile([128, NT], f32, tag="VS")
                nc.vector.tensor_scalar(out=VS[:], in0=SRT[:, :, 7], scalar1=float(COND),
                                        scalar2=None, op0=A.is_gt)

                jleadB = pb.tile([128, CAP], f32, tag="jleadB")
                nc.vector.memset(jleadB[:], 0.0)

                # ---- F: blocked greedy scan
                CID = pw.tile([128, NT], f32, tag="CID")
                LEADC = pw.tile([128, NT], f32, tag="LEADC")

                def ov_block(x1i, x2i, y1i, y2i, a3i, c0, L, tagp):
                    # overlap bool [128, L] of candidates (per-part scalars) vs
                    # sorted rows c0:c0+L (broadcast rows)
                    mnx2 = pk.tile([128, L], f32, tag=tagp + "mnx2")
                    nc.vector.tensor_scalar(out=mnx2[:], in0=x2B[:, c0:c0 + L],
                                            scalar1=x2i, scalar2=None, op0=A.min)
                    ixn = pk.tile([128, L], f32, tag=tagp + "ixn")
                    nc.vector.scalar_tensor_tensor(out=ixn[:], in0=x1B[:, c0:c0 + L],
                                                   scalar=x1i, in1=mnx2[:],
                                                   op0=A.max, op1=A.subtract)
                    ixr = pk.tile([128, L], f32, tag=tagp + "ixr")
                    nc.vector.tensor_scalar(out=ixr[:], in0=ixn[:], scalar1=-1.0,
                                            scalar2=0.0, op0=A.mult, op1=A.max)
                    mny2 = pk.tile([128, L], f32, tag=tagp + "mny2")
                    nc.vector.tensor_scalar(out=mny2[:], in0=y2B[:, c0:c0 + L],
                                            scalar1=y2i, scalar2=None, op0=A.min)
                    iyn = pk.tile([128, L], f32, tag=tagp + "iyn")
                    nc.vector.scalar_tensor_tensor(out=iyn[:], in0=y1B[:, c0:c0 + L],
                                                   scalar=y1i, in1=mny2[:],
                                                   op0=A.max, op1=A.subtract)
                    iyr = pk.tile([128, L], f32, tag=tagp + "iyr")
                    nc.vector.tensor_scalar(out=iyr[:], in0=iyn[:], scalar1=-1.0,
                                            scalar2=0.0, op0=A.mult, op1=A.max)
                    inter = pk.tile([128, L], f32, tag=tagp + "inter")
                    nc.vector.tensor_tensor(out=inter[:], in0=ixr[:], in1=iyr[:], op=A.mult)
                    sp = pk.tile([128, L], f32, tag=tagp + "sp")
                    nc.vector.tensor_scalar(out=sp[:], in0=a3B[:, c0:c0 + L],
                                            scalar1=a3i, scalar2=None, op0=A.add)
                    uu = pk.tile([128, L], f32, tag=tagp + "uu")
                    nc.vector.scalar_tensor_tensor(out=uu[:], in0=inter[:], scalar=-float(IOU),
                                                   in1=sp[:], op0=A.mult, op1=A.add)
                    ovb = pk.tile([128, L], f32, tag=tagp + "ovb")
                    nc.vector.tensor_tensor(out=ovb[:], in0=inter[:], in1=uu[:], op=A.is_gt)
                    return ovb

                for b in range(NBLK):
                    x1i = SRT[:, b, 8:9]
                    x2i = SRT[:, b, 9:10]
                    y1i = SRT[:, b, 10:11]
                    y2i = SRT[:, b, 11:12]
                    a3i = SRT[:, b, 12:13]
                    runmin = pk.tile([128, 1], f32, tag="runmin")
                    nc.vector.memset(runmin[:], 0.0)
                    # ext suppression + cid-base over prefix blocks
                    for c0 in range(0, b * 128, 512):
                        L = min(512, b * 128 - c0)
                        ovb = ov_block(x1i, x2i, y1i, y2i, a3i, c0, L, "g")
                        sel = pk.tile([128, L], f32, tag="esel")
                        nc.vector.tensor_tensor(out=sel[:], in0=ovb[:],
                                                in1=jleadB[:, c0:c0 + L], op=A.mult)
                        cmn = pk.tile([128, 1], f32, tag="cmn")
                        nc.vector.tensor_reduce(out=cmn[:], in_=sel[:], axis=AX, op=A.min)
                        nc.vector.tensor_tensor(out=runmin[:], in0=runmin[:], in1=cmn[:],
                                                op=A.min)
                    supp = pk.tile([128, 1], f32, tag="supp")
                    nc.vector.tensor_scalar(out=supp[:], in0=runmin[:], scalar1=-8192.0,
                                            scalar2=None, op0=A.is_lt)
                    # within-block overlap matrix + strict-lower mask, transposed
                    ovblk = ov_block(x1i, x2i, y1i, y2i, a3i, b * 128, 128, "g")
                    Mb = pk.tile([128, 128], f32, tag="Mb")
                    nc.vector.tensor_tensor(out=Mb[:], in0=ovblk[:], in1=LT128[:], op=A.mult)
                    MbTp = ps.tile([128, 128], f32, tag="MbTp", space="PSUM")
                    nc.tensor.transpose(out=MbTp[:], in_=Mb[:], identity=ident[:])
                    MbT = pk.tile([128, 128], f32, tag="MbT")
                    nc.vector.tensor_copy(out=MbT[:], in_=MbTp[:])
                    # rounds
                    notsupp = pk.tile([128, 1], f32, tag="notsupp")
                    nc.vector.tensor_scalar(out=notsupp[:], in0=supp[:], scalar1=-1.0,
                                            scalar2=1.0, op0=A.mult, op1=A.add)
                    u = pk.tile([128, 1], f32, tag="u")
                    nc.vector.tensor_tensor(out=u[:], in0=VS[:, b:b + 1], in1=notsupp[:],
                                            op=A.mult)
                    lvec = pk.tile([128, 1], f32, tag="lvec")
                    nc.vector.memset(lvec[:], 0.0)
                    for r in range(RROUNDS):
                        p1 = ps1.tile([128, 1], f32, tag="p1", space="PSUM")
                        nc.tensor.matmul(out=p1[:], lhsT=MbT[:], rhs=u[:],
                                         start=True, stop=True)
                        e1 = pk.tile([128, 1], f32, tag="e1")
                        nc.vector.tensor_scalar(out=e1[:], in0=p1[:], scalar1=0.0,
                                                scalar2=None, op0=A.is_equal)
                        newl = pk.tile([128, 1], f32, tag="newl")
                        nc.vector.tensor_tensor(out=newl[:], in0=u[:], in1=e1[:], op=A.mult)
                        nc.vector.tensor_tensor(out=lvec[:], in0=lvec[:], in1=newl[:],
                                                op=A.max)
                        if r < RROUNDS - 1:
                            p2 = ps1.tile([128, 1], f32, tag="p2", space="PSUM")
                            nc.tensor.matmul(out=p2[:], lhsT=MbT[:], rhs=newl[:],
                                             start=True, stop=True)
                            nd = pk.tile([128, 1], f32, tag="nd")
                            nc.vector.tensor_scalar(out=nd[:], in0=p2[:], scalar1=0.0,
                                                    scalar2=None, op0=A.is_equal)
                            nn = pk.tile([128, 1], f32, tag="nn")
                            nc.vector.tensor_scalar(out=nn[:], in0=newl[:], scalar1=-1.0,
                                                    scalar2=1.0, op0=A.mult, op1=A.add)
                            nc.vector.tensor_tensor(out=u[:], in0=u[:], in1=nd[:], op=A.mult)
                            nc.vector.tensor_tensor(out=u[:], in0=u[:], in1=nn[:], op=A.mult)
                    nc.vector.tensor_copy(out=LEADC[:, b:b + 1], in_=lvec[:])
                    # jlead segment: lvec * (i - 16384), broadcast to all partitions
                    iidx = pk.tile([128, 1], f32, tag="iidx")
                    nc.vector.tensor_scalar(out=iidx[:], in0=IOC[:], scalar1=float(b * 128),
                                            scalar2=-float(S16K), op0=A.add, op1=A.add)
                    jlv = pk.tile([128, 1], f32, tag="jlv")
                    nc.vector.tensor_tensor(out=jlv[:], in0=lvec[:], in1=iidx[:], op=A.mult)
                    nc.gpsimd.dma_start(out=coljl[s, b].rearrange("(p a) -> p a", a=1), in_=jlv[:])
                    nc.gpsimd.dma_start(out=jleadB[:, b * 128:(b + 1) * 128],
                                        in_=bcast(coljl[s, b], 128))
                    # local cid: min over in-block leaders overlapping i
                    lsel = pk.tile([128, 128], f32, tag="lsel")
                    nc.vector.tensor_tensor(out=lsel[:], in0=ovblk[:],
                                            in1=jleadB[:, b * 128:(b + 1) * 128], op=A.mult)
                    lmn = pk.tile([128, 1], f32, tag="lmn")
                    nc.vector.tensor_reduce(out=lmn[:], in_=lsel[:], axis=AX, op=A.min)
                    cmin = pk.tile([128, 1], f32, tag="cmin")
                    nc.vector.tensor_tensor(out=cmin[:], in0=runmin[:], in1=lmn[:], op=A.min)
                    cidv = pk.tile([128, 1], f32, tag="cidv")
                    nc.vector.tensor_scalar(out=cidv[:], in0=cmin[:], scalar1=float(S16K),
                                            scalar2=None, op0=A.add)
                    okc = pk.tile([128, 1], f32, tag="okc")
                    nc.vector.tensor_scalar(out=okc[:], in0=cidv[:], scalar1=8192.0,
                                            scalar2=None, op0=A.is_lt)
                    nc.vector.tensor_tensor(out=okc[:], in0=okc[:], in1=VS[:, b:b + 1],
                                            op=A.mult)
                    ok8 = pk.tile([128, 1], i8, tag="ok8")
                    nc.vector.tensor_copy(out=ok8[:], in_=okc[:])
                    nc.vector.select(out=CID[:, b:b + 1], mask=ok8[:], on_true=cidv[:],
                                     on_false=MONE[:])

                # ---- G: cluster rank (count of earlier members of same cluster)
                nc.gpsimd.dma_start(out=colbuf[s, 6].rearrange("(t p) -> p t", p=128),
                                    in_=CID[:])
                cidB = pb.tile([128, CAP], f32, tag="cidB")
                nc.gpsimd.dma_start(out=cidB[:], in_=bcast(colbuf[s, 6], CAP))
                RK = pw.tile([128, NT], f32, tag="RK")
                for t in range(NT):
                    iidx = pk.tile([128, 1], f32, tag="iidx")
                    nc.vector.tensor_scalar(out=iidx[:], in0=IOC[:], scalar1=float(t * 128),
                                            scalar2=None, op0=A.add)
                    nch = (t * 128 + 128 + 511) // 512
                    acc = None
                    for c in range(nch):
                        c0 = c * 512
                        L = min(512, CAP - c0)
                        lt = pk.tile([128, L], f32, tag="lt")
                        nc.vector.tensor_scalar(out=lt[:], in0=JROW[:, c0:c0 + L],
                                                scalar1=iidx[:, :1], scalar2=None, op0=A.is_lt)
                        part = pk.tile([128, 1], f32, tag="part")
                        nc.vector.scalar_tensor_tensor(out=junk[:, 0:L], in0=cidB[:, c0:c0 + L],
                                                       scalar=CID[:, t:t + 1], in1=lt[:],
                                                       op0=A.is_equal, op1=A.mult,
                                                       accum_out=part[:])
                        if acc is None:
                            acc = pk.tile([128, 1], f32, tag="acc")
                            nc.vector.tensor_copy(out=acc[:], in_=part[:])
                        else:
                            nc.vector.tensor_tensor(out=acc[:], in0=acc[:], in1=part[:], op=A.add)
                    nc.vector.tensor_copy(out=RK[:, t:t + 1], in_=acc[:])

                # ---- H: cluster-table scatter
                DST = pw.tile([128, NT], f32, tag="DST")
                for t in range(NT):
                    m15 = pk.tile([128, 1], f32, tag="m15")
                    nc.vector.tensor_scalar(out=m15[:], in0=RK[:, t:t + 1], scalar1=15.0,
                                            scalar2=None, op0=A.min)
                    d0 = pk.tile([128, 1], f32, tag="d0")
                    nc.vector.scalar_tensor_tensor(out=d0[:], in0=CID[:, t:t + 1],
                                                   scalar=16.0, in1=m15[:],
                                                   op0=A.mult, op1=A.add)
                    c1 = pk.tile([128, 1], f32, tag="c1")
                    nc.vector.tensor_scalar(out=c1[:], in0=CID[:, t:t + 1], scalar1=0.0,
                                            scalar2=None, op0=A.is_ge)
                    c2 = pk.tile([128, 1], f32, tag="c2")
                    nc.vector.tensor_scalar(out=c2[:], in0=RK[:, t:t + 1], scalar1=16.0,
                                            scalar2=None, op0=A.is_lt)
                    nc.vector.tensor_tensor(out=c1[:], in0=c1[:], in1=c2[:], op=A.mult)
                    c18 = pk.tile([128, 1], i8, tag="c18")
                    nc.vector.tensor_copy(out=c18[:], in_=c1[:])
                    nc.vector.select(out=DST[:, t:t + 1], mask=c18[:], on_true=d0[:],
                                     on_false=DUMPC[:])
                DSTi = pw.tile([128, NT], i32, tag="DSTi")
                nc.vector.tensor_copy(out=DSTi[:], in_=DST[:])
                for t in range(NT):
                    nc.gpsimd.indirect_dma_start(
                        out=ctab[s][:], in_=SRT[:, t, 0:7],
                        out_offset=bass.IndirectOffsetOnAxis(ap=DSTi[:, t:t + 1], axis=0),
                        in_offset=None)

                # ---- I: MLP + softmax + merge
                CL = pw.tile([128, NT, 112], f32, tag="CL")
                nc.gpsimd.dma_start(
                    out=CL[:],
                    in_=ctab[s][0:NT * 128 * 16].rearrange(
                        "(t p sl) c -> p t (sl c)", p=128, sl=16))
                CLT = pb.tile([112, CAP], f32, tag="CLT")
                for t in range(NT):
                    tp = ps.tile([112, 128], f32, tag="MbTp", name="tp", space="PSUM")
                    nc.tensor.transpose(out=tp[:], in_=CL[:, t, :], identity=ident[:])
                    nc.vector.tensor_copy(out=CLT[:, t * 128:(t + 1) * 128], in_=tp[:])
                LG = pw.tile([128, NT, 64], f32, tag="LG")
                for cc in range(3):
                    c0 = cc * 512
                    hs = pk.tile([128, 512], f32, tag="hs")
                    logitsP = ps.tile([16, 512], f32, tag="logitsP", space="PSUM")
                    absP = ps.tile([16, 512], f32, tag="absP", space="PSUM")
                    for bb in range(8):
                        accp = ps1.tile([128, 512], f32, tag="accp", space="PSUM")
                        nc.tensor.matmul(out=accp[:], lhsT=W1T[bb][:],
                                         rhs=CLT[:, c0:c0 + 512], start=True, stop=True)
                        nc.scalar.activation(out=hs[:], in_=accp[:], func=AF.Relu,
                                             bias=B1C[:, :1], scale=1.0)
                        nc.tensor.matmul(out=logitsP[:], lhsT=W2T[bb][:], rhs=hs[:],
                                         start=(bb == 0), stop=(bb == 7))
                    clabs = pk.tile([112, 512], f32, tag="clabs")
                    nc.vector.scalar_tensor_tensor(out=clabs[:], in0=CLT[:, c0:c0 + 512],
                                                   scalar=-1.0, in1=CLT[:, c0:c0 + 512],
                                                   op0=A.mult, op1=A.max)
                    nc.tensor.matmul(out=absP[:], lhsT=ONESBD[:], rhs=clabs[:],
                                     start=True, stop=True)
                    LGS = pk.tile([128, 512], f32, tag="LGS")
                    nc.vector.tensor_scalar(out=LGS[0:16, :], in0=logitsP[:],
                                            scalar1=B2C[:, :1], scalar2=None, op0=A.add)
                    nc.vector.tensor_copy(out=LGS[32:48, :], in_=absP[:])
                    for q in range(4):
                        tp2 = ps.tile([128, 128], f32, tag="MbTp", name="tp2", space="PSUM")
                        nc.tensor.transpose(out=tp2[:], in_=LGS[:, q * 128:(q + 1) * 128],
                                            identity=ident[:])
                        nc.vector.tensor_copy(out=LG[:, cc * 4 + q, :], in_=tp2[:, 0:64])
                INF = pw.tile([128, NT, 9], f32, tag="INF")
                for t in range(NT):
                    lg = LG[:, t, 0:16]
                    ab = LG[:, t, 32:48]
                    okf = pk.tile([128, 16], f32, tag="okf")
                    nc.vector.tensor_scalar(out=okf[:], in0=ab[:], scalar1=0.0,
                                            scalar2=None, op0=A.is_gt)
                    okb = pk.tile([128, 16], i8, tag="okb")
                    nc.vector.tensor_copy(out=okb[:], in_=okf[:])
                    lm = pk.tile([128, 16], f32, tag="lm")
                    nc.vector.select(out=lm[:], mask=okb[:], on_true=lg[:], on_false=NEG9[:])
                    mx = pk.tile([128, 1], f32, tag="mx")
                    nc.vector.tensor_reduce(out=mx[:], in_=lm[:], axis=AX, op=A.max)
                    sh = pk.tile([128, 16], f32, tag="sh")
                    nc.vector.tensor_scalar(out=sh[:], in0=lm[:], scalar1=mx[:, :1],
                                            scalar2=None, op0=A.subtract)
                    ex = pk.tile([128, 16], f32, tag="ex")
                    nc.scalar.activation(out=ex[:], in_=sh[:], func=AF.Exp, bias=0.0, scale=1.0)
                    sm = pk.tile([128, 1], f32, tag="sm")
                    nc.vector.tensor_reduce(out=sm[:], in_=ex[:], axis=AX, op=A.add)
                    rcs = pk.tile([128, 1], f32, tag="rcs")
                    nc.vector.reciprocal(out=rcs[:], in_=sm[:])
                    wgt = pk.tile([128, 16], f32, tag="wgt")
                    nc.vector.tensor_scalar(out=wgt[:], in0=ex[:], scalar1=rcs[:, :1],
                                            scalar2=None, op0=A.mult)
                    WX = pk.tile([128, 112], f32, tag="WX")
                    wx3 = WX[:].rearrange("p (s d) -> p s d", d=7)
                    for d in range(7):
                        nc.vector.tensor_copy(out=wx3[:, :, d], in_=wgt[:])
                    wq = pk.tile([128, 112], f32, tag="wq")
                    nc.vector.tensor_tensor(out=wq[:], in0=CL[:, t, :], in1=WX[:], op=A.mult)
                    f56 = pk.tile([128, 56], f32, tag="f56")
                    nc.vector.tensor_tensor(out=f56[:], in0=wq[:, 0:56], in1=wq[:, 56:112],
                                            op=A.add)
                    f28 = pk.tile([128, 28], f32, tag="f28")
                    nc.vector.tensor_tensor(out=f28[:], in0=f56[:, 0:28], in1=f56[:, 28:56],
                                            op=A.add)
                    f14 = pk.tile([128, 14], f32, tag="f14")
                    nc.vector.tensor_tensor(out=f14[:], in0=f28[:, 0:14], in1=f28[:, 14:28],
                                            op=A.add)
                    mg = pk.tile([128, 7], f32, tag="mg")
                    nc.vector.tensor_tensor(out=mg[:], in0=f14[:, 0:7], in1=f14[:, 7:14],
                                            op=A.add)
                    # size fallback
                    sneg = pk.tile([128, 3], f32, tag="sneg")
                    nc.vector.tensor_scalar(out=sneg[:], in0=mg[:, 3:6], scalar1=0.0,
                                            scalar2=None, op0=A.is_le)
                    sn8 = pk.tile([128, 3], i8, tag="sn8")
                    nc.vector.tensor_copy(out=sn8[:], in_=sneg[:])
                    szf = pk.tile([128, 3], f32, tag="szf")
                    nc.vector.select(out=szf[:], mask=sn8[:], on_true=SRT[:, t, 3:6],
                                     on_false=mg[:, 3:6])
                    nc.vector.tensor_copy(out=INF[:, t, 0:3], in_=mg[:, 0:3])
                    nc.vector.tensor_copy(out=INF[:, t, 3:6], in_=szf[:])
                    nc.vector.tensor_copy(out=INF[:, t, 6:7], in_=mg[:, 6:7])
                    nc.vector.tensor_copy(out=INF[:, t, 7:8], in_=SRT[:, t, 13:14])
                    nc.vector.tensor_scalar(out=INF[:, t, 8:9], in0=CLABc[:], scalar1=0.0,
                                            scalar2=None, op0=A.add)
                    nc.vector.tensor_scalar(out=INF[:, t, :], in0=INF[:, t, :],
                                            scalar1=LEADC[:, t:t + 1], scalar2=None,
                                            op0=A.mult)

                # ---- J: scatter to global rows
                if dbgx is not None:
                    DBG = pw.tile([128, 32], f32, tag="DBG")
                    nc.vector.tensor_copy(out=DBG[:, 0:12], in_=SRT[:, :, 14])
                    nc.vector.tensor_copy(out=DBG[:, 12:24], in_=LEADC[:])
                    nc.vector.tensor_copy(out=DBG[:, 24:32], in_=INF[:, 0, 0:8])
                    nc.sync.dma_start(out=dbgx[s], in_=DBG[:])
                # invalid rows (key<=COND, incl. zero pads) -> OOB index so the
                # bounds check skips them instead of clobbering real rows
                vs8 = pw.tile([128, NT], i8, tag="vs8")
                nc.vector.tensor_copy(out=vs8[:], in_=VS[:])
                OOBC = pw.tile([128, NT], f32, tag="OOBC")
                nc.vector.memset(OOBC[:], float(N))
                GSf = pw.tile([128, NT], f32, tag="GSf")
                nc.vector.select(out=GSf[:], mask=vs8[:], on_true=SRT[:, :, 14],
                                 on_false=OOBC[:])
                GS = pw.tile([128, NT], i32, tag="GS")
                nc.vector.tensor_copy(out=GS[:], in_=GSf[:])
                for t in range(NT):
                    nc.gpsimd.indirect_dma_start(
                        out=oinfo[s][:], in_=INF[:, t, :],
                        out_offset=bass.IndirectOffsetOnAxis(ap=GS[:, t:t + 1], axis=0),
                        in_offset=None, bounds_check=bcreg, oob_is_err=False)
                    nc.gpsimd.indirect_dma_start(
                        out=olead[s][:].rearrange("(n a) -> n a", a=1),
                        in_=LEADC[:, t:t + 1],
                        out_offset=bass.IndirectOffsetOnAxis(ap=GS[:, t:t + 1], axis=0),
                        in_offset=None, bounds_check=bcreg, oob_is_err=False)

    return nc


def _prep_weights(W1, b1, W2, b2):
    w1bd = np.zeros((8, 112, 128), np.float32)
    w2bd = np.zeros((8, 128, 16), np.float32)
    for bb in range(8):
        for sp in range(2):
            s = 2 * bb + sp
            w1bd[bb, 7 * s:7 * s + 7, 64 * sp:64 * sp + 64] = W1
            w2bd[bb, 64 * sp:64 * sp + 64, s] = W2[:, 0]
    onesbd = np.zeros((112, 16), np.float32)
    for s in range(16):
        onesbd[7 * s:7 * s + 7, s] = 1.0
    b1c = np.concatenate([b1, b1]).astype(np.float32).reshape(128, 1)
    b2c = np.full((16, 1), np.float32(b2[0]), np.float32)
    return w1bd, w2bd, onesbd, b1c, b2c


def kernel(boxes, scores, labels, W1, b1, W2, b2):
    from concourse.bass_utils import run_bass_kernel_spmd

    boxes = np.asarray(boxes, np.float32)
    scores = np.asarray(scores, np.float32)
    labels = np.asarray(labels)
    w1bd, w2bd, onesbd, b1c, b2c = _prep_weights(
        np.asarray(W1, np.float32), np.asarray(b1, np.float32),
        np.asarray(W2, np.float32), np.asarray(b2, np.float32))

    # shard (frame, class) -> (core, slot)
    shards = [(f, c) for f in range(B) for c in range(3)]
    place = {}
    for k in range(8):
        place[(k, 0)] = shards[k]
        if 8 + k < len(shards):
            place[(k, 1)] = shards[8 + k]

    in_maps = []
    for k in range(8):
        m = {"w1bd": w1bd, "w2bd": w2bd, "onesbd": onesbd, "b1c": b1c, "b2c": b2c}
        cb = np.zeros((2, CAP, 7), np.float32)
        cs = np.zeros((2, CAP), np.float32)
        co = np.full((2, CAP), PADORIG, np.float32)
        fs = np.zeros((2, N), np.float32)
        cl = np.zeros((2, 128), np.float32)
        for slot in range(2):
            if (k, slot) in place:
                f, c = place[(k, slot)]
                sel = np.where(labels[f] == c)[0]
                ncl = len(sel)
                assert ncl <= CAP, f"class count {ncl} exceeds CAP {CAP}"
                cb[slot, :ncl] = boxes[f, sel]
                cs[slot, :ncl] = scores[f, sel]
                co[slot, :ncl] = sel.astype(np.float32)
                fs[slot] = scores[f]
                cl[slot, :] = np.float32(c)
        m.update(cboxes=cb, cscore=cs, corig=co, fscore=fs, clab=cl)
        in_maps.append(m)

    nc = _cache.get("nc")
    if nc is None:
        nc = _build_program()
        _cache["nc"] = nc

    res = run_bass_kernel_spmd(nc, in_maps, list(range(8)))

    info = np.zeros((B, N, 9), np.float32)
    lead = np.zeros((B, N), bool)
    for (k, slot), (f, c) in place.items():
        info[f] += res.results[k][f"oinfo{slot}"]
        lead[f] |= res.results[k][f"olead{slot}"] > 0.5
    return info, lead


# revision 14
# speedup vs baseline: 1.9104x; 1.9104x over previous
"""Trainium2 Bass kernel for nn_Ensemble_51058571214927 (nms_detection).

Sharding: 12 (frame, class) shards over 8 cores x 2 slots (SPMD).
Device does: exact stable rank (score desc, index asc), scatter-sort,
blocked greedy NMS clustering, cluster-rank, cluster-table scatter,
merge MLP + softmax + weighted merge, scatter to global sorted rows.
Host does: class compaction (sharding), weight layout prep, shard-sum merge.
"""
import numpy as np

# ---------------------------------------------------------------- constants
B, N = 4, 4096
CAP = 1536            # per-class capacity (max observed 1429)
NT = CAP // 128       # 12 member tiles
NBLK = CAP // 128     # 12 scan blocks
MAXC = 16
DUMP = CAP * MAXC     # dump row in cluster table
CTROWS = 24704        # cluster table rows (>= DUMP+16, = 128*193; *7 = 1544*112)
COND = np.float32(0.2)
IOU = np.float32(0.3)
NEG = np.float32(-1e30)
CTIE = np.float32(1e12)
S16K = np.float32(16384.0)
RROUNDS = 4
PADORIG = np.float32(5000.0)

_cache = {}


def _build_program():
    import bir_fixups  # noqa: F401  (toolchain workarounds)
    import concourse.bass as bass
    import concourse.mybir as mybir
    from concourse.tile import TileContext
    from concourse.masks import make_identity

    f32, i32, i8 = mybir.dt.float32, mybir.dt.int32, mybir.dt.int8
    A = mybir.AluOpType
    AF = mybir.ActivationFunctionType
    AX = mybir.AxisListType.X

    nc = bass.Bass()

    # ------------------------------------------------------------- tensors
    cboxes = nc.dram_tensor("cboxes", [2, CAP, 7], f32, kind="ExternalInput")
    cscore = nc.dram_tensor("cscore", [2, CAP], f32, kind="ExternalInput")
    corig = nc.dram_tensor("corig", [2, CAP], f32, kind="ExternalInput")
    fscore = nc.dram_tensor("fscore", [2, N], f32, kind="ExternalInput")
    clab = nc.dram_tensor("clab", [2, 128], f32, kind="ExternalInput")
    w1bd = nc.dram_tensor("w1bd", [8, 112, 128], f32, kind="ExternalInput")
    w2bd = nc.dram_tensor("w2bd", [8, 128, 16], f32, kind="ExternalInput")
    onesbd_d = nc.dram_tensor("onesbd", [112, 16], f32, kind="ExternalInput")
    b1c_d = nc.dram_tensor("b1c", [128, 1], f32, kind="ExternalInput")
    b2c_d = nc.dram_tensor("b2c", [16, 1], f32, kind="ExternalInput")

    import os
    _dbg = "ExternalOutput" if os.environ.get("KDBG") else "Internal"
    sortedtab = [nc.dram_tensor(f"sortedtab{s}", [CAP + 128, 16], f32, kind=_dbg)
                 for s in range(2)]
    colbuf = nc.dram_tensor("colbuf", [2, 8, CAP], f32, kind=_dbg)   # key,x1,x2,y1,y2,a3,cid
    coljl = nc.dram_tensor("coljl", [2, NBLK, 128], f32)
    ctab = [nc.dram_tensor(f"ctab{s}", [CTROWS, 7], f32, kind=_dbg) for s in range(2)]

    oinfo = [nc.dram_tensor(f"oinfo{s}", [N, 9], f32, kind="ExternalOutput")
             for s in range(2)]
    dbgx = nc.dram_tensor("dbgx", [2, 128, 32], f32, kind=_dbg) if _dbg == "ExternalOutput" else None
    olead = [nc.dram_tensor(f"olead{s}", [N], f32, kind="ExternalOutput")
             for s in range(2)]

    def bcast(ap_1d, count):
        # DRAM 1-D region -> [128, count] partition broadcast AP
        return bass.AP(ap_1d.tensor, ap_1d.offset, [[0, 128], [1, count]])

    with TileContext(nc) as tc:
        with tc.tile_pool(name="pc", bufs=1) as pc, \
             tc.tile_pool(name="pb", bufs=1) as pb, \
             tc.tile_pool(name="pw", bufs=1) as pw, \
             tc.tile_pool(name="pk", bufs=1) as pk, \
             tc.tile_pool(name="ps", bufs=1, space="PSUM") as ps, \
             tc.tile_pool(name="ps1", bufs=1, space="PSUM") as ps1:

            # ---------------------------------------------------- constants
            ident = pc.tile([128, 128], f32, tag="ident")
            make_identity(nc, ident[:])
            IOC = pc.tile([128, 1], f32, tag="IOC")
            nc.gpsimd.iota(IOC[:], pattern=[[0, 1]], base=0, channel_multiplier=1,
                           allow_small_or_imprecise_dtypes=True)
            JROW = pc.tile([128, CAP], f32, tag="JROW")
            nc.gpsimd.iota(JROW[:], pattern=[[1, CAP]], base=0, channel_multiplier=0,
                           allow_small_or_imprecise_dtypes=True)
            FORIG = pc.tile([128, N], f32, tag="FORIG")
            nc.gpsimd.iota(FORIG[:], pattern=[[1, N]], base=0, channel_multiplier=0,
                           allow_small_or_imprecise_dtypes=True)
            LT128 = pc.tile([128, 128], f32, tag="LT128")
            nc.vector.tensor_scalar(out=LT128[:], in0=JROW[:, 0:128], scalar1=IOC[:, :1],
                                    scalar2=None, op0=A.is_lt)
            NEGBIG = pc.tile([128, 512], f32, tag="NEGBIG")
            nc.vector.memset(NEGBIG[:], float(NEG))
            NEG9 = pc.tile([128, 16], f32, tag="NEG9")
            nc.vector.memset(NEG9[:], -1e9)
            MONE = pc.tile([128, 1], f32, tag="MONE")
            nc.vector.memset(MONE[:], -1.0)
            DUMPC = pc.tile([128, 1], f32, tag="DUMPC")
            nc.vector.memset(DUMPC[:], float(DUMP))
            ZER16 = pc.tile([128, 16], f32, tag="ZER16")
            nc.vector.memset(ZER16[:], 0.0)
            ZBIG = pc.tile([128, 1408], f32, tag="ZBIG")
            nc.vector.memset(ZBIG[:], 0.0)
            W1T = [pc.tile([112, 128], f32, tag=f"W1T{i}", name=f"W1T{i}") for i in range(8)]
            for i in range(8):
                nc.sync.dma_start(out=W1T[i][:], in_=w1bd[i])
            W2T = [pc.tile([128, 16], f32, tag=f"W2T{i}", name=f"W2T{i}") for i in range(8)]
            for i in range(8):
                nc.sync.dma_start(out=W2T[i][:], in_=w2bd[i])
            ONESBD = pc.tile([112, 16], f32, tag="ONESBD")
            nc.sync.dma_start(out=ONESBD[:], in_=onesbd_d[:])
            B1C = pc.tile([128, 1], f32, tag="B1C")
            nc.sync.dma_start(out=B1C[:], in_=b1c_d[:])
            B2C = pc.tile([16, 1], f32, tag="B2C")
            nc.sync.dma_start(out=B2C[:], in_=b2c_d[:])

            bcreg = nc.gpsimd.alloc_register("bcreg")
            nc.gpsimd.reg_mov(bcreg, N - 1)

            # zero outputs/scratch with partition-contiguous APs (few descriptors)
            def zfill(handle, nelem):
                per = nelem // 128
                assert per * 128 == nelem and per <= 1408
                nc.gpsimd.dma_start(
                    out=bass.AP(handle[:].tensor, 0, [[per, 128], [1, per]]),
                    in_=ZBIG[:, 0:per])
            for s in range(2):
                zfill(oinfo[s], N * 9)
                zfill(olead[s], N)
                zfill(sortedtab[s], (CAP + 128) * 16)
                zfill(ctab[s], CTROWS * 7)

            # ================================================= per-slot body
            for s in range(2):
                # ---- A: load member-major, geometry, key
                BX = pw.tile([128, NT, 7], f32, tag="BX")
                nc.sync.dma_start(out=BX[:], in_=cboxes[s].rearrange("(t p) c -> p t c", p=128))
                SC = pw.tile([128, NT], f32, tag="SC")
                nc.sync.dma_start(out=SC[:], in_=cscore[s].rearrange("(t p) -> p t", p=128))
                OG = pw.tile([128, NT], f32, tag="OG")
                nc.sync.dma_start(out=OG[:], in_=corig[s].rearrange("(t p) -> p t", p=128))
                CLABc = pw.tile([128, 1], f32, tag="CLABc")
                nc.sync.dma_start(out=CLABc[:], in_=clab[s].rearrange("(p a) -> p a", a=1))

                v01 = pw.tile([128, NT], f32, tag="v01")
                nc.vector.tensor_scalar(out=v01[:], in0=SC[:], scalar1=float(COND),
                                        scalar2=None, op0=A.is_gt)
                v8 = pw.tile([128, NT], i8, tag="v8")
                nc.vector.tensor_copy(out=v8[:], in_=v01[:])
                KEY = pw.tile([128, NT], f32, tag="KEY")
                nc.vector.select(out=KEY[:], mask=v8[:], on_true=SC[:],
                                 on_false=NEGBIG[:, 0:NT])

                OFFc = pw.tile([128, 1], f32, tag="OFFc")
                nc.vector.tensor_scalar(out=OFFc[:], in0=CLABc[:], scalar1=1e4,
                                        scalar2=None, op0=A.mult)
                cx = pw.tile([128, NT], f32, tag="cx")
                nc.vector.tensor_scalar(out=cx[:], in0=BX[:, :, 0], scalar1=OFFc[:, :1],
                                        scalar2=None, op0=A.add)
                hx = pw.tile([128, NT], f32, tag="hx")
                nc.vector.tensor_scalar(out=hx[:], in0=BX[:, :, 3], scalar1=0.5,
                                        scalar2=None, op0=A.mult)
                hy = pw.tile([128, NT], f32, tag="hy")
                nc.vector.tensor_scalar(out=hy[:], in0=BX[:, :, 4], scalar1=0.5,
                                        scalar2=None, op0=A.mult)
                gx1 = pw.tile([128, NT], f32, tag="gx1")
                nc.vector.tensor_tensor(out=gx1[:], in0=cx[:], in1=hx[:], op=A.subtract)
                gx2 = pw.tile([128, NT], f32, tag="gx2")
                nc.vector.tensor_tensor(out=gx2[:], in0=cx[:], in1=hx[:], op=A.add)
                gy1 = pw.tile([128, NT], f32, tag="gy1")
                nc.vector.tensor_tensor(out=gy1[:], in0=BX[:, :, 1], in1=hy[:], op=A.subtract)
                gy2 = pw.tile([128, NT], f32, tag="gy2")
                nc.vector.tensor_tensor(out=gy2[:], in0=BX[:, :, 1], in1=hy[:], op=A.add)
                wd = pw.tile([128, NT], f32, tag="wd")
                nc.vector.tensor_tensor(out=wd[:], in0=gx2[:], in1=gx1[:], op=A.subtract)
                hg = pw.tile([128, NT], f32, tag="hg")
                nc.vector.tensor_tensor(out=hg[:], in0=gy2[:], in1=gy1[:], op=A.subtract)
                ar = pw.tile([128, NT], f32, tag="ar")
                nc.vector.tensor_tensor(out=ar[:], in0=wd[:], in1=hg[:], op=A.mult)
                a3 = pw.tile([128, NT], f32, tag="a3")
                nc.vector.tensor_scalar(out=a3[:], in0=ar[:], scalar1=float(IOU),
                                        scalar2=None, op0=A.mult)

                # ---- B: class rank (key desc, orig asc)
                nc.gpsimd.dma_start(out=colbuf[s, 0].rearrange("(t p) -> p t", p=128),
                                    in_=KEY[:])
                keyB = pb.tile([128, CAP], f32, tag="keyB")
                nc.gpsimd.dma_start(out=keyB[:], in_=bcast(colbuf[s, 0], CAP))
                origB = pb.tile([128, CAP], f32, tag="origB")
                nc.gpsimd.dma_start(out=origB[:], in_=bcast(corig[s], CAP))

                CR = pw.tile([128, NT], f32, tag="CR")
                junk = pk.tile([128, 512], f32, tag="junk")
                for t in range(NT):
                    acc = None
                    for c0 in range(0, CAP, 512):
                        t1 = pk.tile([128, 512], f32, tag="t1")
                        nc.vector.tensor_scalar(out=t1[:], in0=keyB[:, c0:c0 + 512],
                                                scalar1=KEY[:, t:t + 1], scalar2=float(CTIE),
                                                op0=A.subtract, op1=A.mult)
                        r1 = pk.tile([128, 512], f32, tag="r1")
                        nc.vector.tensor_scalar(out=r1[:], in0=origB[:, c0:c0 + 512],
                                                scalar1=OG[:, t:t + 1], scalar2=None,
                                                op0=A.subtract)
                        part = pk.tile([128, 1], f32, tag="part")
                        nc.vector.scalar_tensor_tensor(out=junk[:], in0=t1[:], scalar=0.0,
                                                       in1=r1[:], op0=A.add, op1=A.is_gt,
                                                       accum_out=part[:])
                        if acc is None:
                            acc = pk.tile([128, 1], f32, tag="acc")
                            nc.vector.tensor_copy(out=acc[:], in_=part[:])
                        else:
                            nc.vector.tensor_tensor(out=acc[:], in0=acc[:], in1=part[:], op=A.add)
                    nc.vector.tensor_copy(out=CR[:, t:t + 1], in_=acc[:])

                # ---- C: global rank g
                fkB = pb.tile([128, N], f32, tag="fkB")
                for c0 in range(0, N, 512):
                    fsc = pk.tile([128, 512], f32, tag="fsc")
                    nc.gpsimd.dma_start(out=fsc[:], in_=bcast(fscore[s, c0:c0 + 512], 512))
                    fv = pk.tile([128, 512], f32, tag="fv")
                    nc.vector.tensor_scalar(out=fv[:], in0=fsc[:],
                                            scalar1=float(COND), scalar2=None, op0=A.is_gt)
                    fv8 = pk.tile([128, 512], i8, tag="fv8")
                    nc.vector.tensor_copy(out=fv8[:], in_=fv[:])
                    nc.vector.select(out=fkB[:, c0:c0 + 512], mask=fv8[:],
                                     on_true=fsc[:], on_false=NEGBIG[:])
                G = pw.tile([128, NT], f32, tag="G")
                for t in range(NT):
                    acc = None
                    for c0 in range(0, N, 512):
                        t1 = pk.tile([128, 512], f32, tag="t1")
                        nc.vector.tensor_scalar(out=t1[:], in0=fkB[:, c0:c0 + 512],
                                                scalar1=KEY[:, t:t + 1], scalar2=float(CTIE),
                                                op0=A.subtract, op1=A.mult)
                        r1 = pk.tile([128, 512], f32, tag="r1")
                        nc.vector.tensor_scalar(out=r1[:], in0=FORIG[:, c0:c0 + 512],
                                                scalar1=OG[:, t:t + 1], scalar2=None,
                                                op0=A.subtract)
                        part = pk.tile([128, 1], f32, tag="part")
                        nc.vector.scalar_tensor_tensor(out=junk[:], in0=t1[:], scalar=0.0,
                                                       in1=r1[:], op0=A.add, op1=A.is_gt,
                                                       accum_out=part[:])
                        if acc is None:
                            acc = pk.tile([128, 1], f32, tag="acc")
                            nc.vector.tensor_copy(out=acc[:], in_=part[:])
                        else:
                            nc.vector.tensor_tensor(out=acc[:], in0=acc[:], in1=part[:], op=A.add)
                    nc.vector.tensor_copy(out=G[:, t:t + 1], in_=acc[:])

                # ---- D: stage + scatter to sorted order
                ST = pw.tile([128, NT, 16], f32, tag="STG")
                nc.vector.tensor_copy(out=ST[:, :, 0:7], in_=BX[:])
                nc.vector.tensor_copy(out=ST[:, :, 7], in_=KEY[:])
                nc.vector.tensor_copy(out=ST[:, :, 8], in_=gx1[:])
                nc.vector.tensor_copy(out=ST[:, :, 9], in_=gx2[:])
                nc.vector.tensor_copy(out=ST[:, :, 10], in_=gy1[:])
                nc.vector.tensor_copy(out=ST[:, :, 11], in_=gy2[:])
                nc.vector.tensor_copy(out=ST[:, :, 12], in_=a3[:])
                nc.vector.tensor_copy(out=ST[:, :, 13], in_=SC[:])
                nc.vector.tensor_copy(out=ST[:, :, 14], in_=G[:])
                nc.vector.tensor_copy(out=ST[:, :, 15], in_=CR[:])
                CRi = pw.tile([128, NT], i32, tag="CRi")
                nc.vector.tensor_copy(out=CRi[:], in_=CR[:])
                for t in range(NT):
                    nc.gpsimd.indirect_dma_start(
                        out=sortedtab[s][:], in_=ST[:, t, :],
                        out_offset=bass.IndirectOffsetOnAxis(ap=CRi[:, t:t + 1], axis=0),
                        in_offset=None)

                # ---- E: reload sorted + broadcast geometry rows
                SRT = pw.tile([128, NT, 16], f32, tag="SRT")
                nc.gpsimd.dma_start(
                    out=SRT[:], in_=sortedtab[s][0:CAP].rearrange("(t p) c -> p t c", p=128))
                for ci, col in ((1, 8), (2, 9), (3, 10), (4, 11), (5, 12)):
                    nc.gpsimd.dma_start(out=colbuf[s, ci].rearrange("(t p) -> p t", p=128),
                                        in_=SRT[:, :, col])
                x1B = pb.tile([128, CAP], f32, tag="x1B")
                nc.gpsimd.dma_start(out=x1B[:], in_=bcast(colbuf[s, 1], CAP))
                x2B = pb.tile([128, CAP], f32, tag="x2B")
                nc.gpsimd.dma_start(out=x2B[:], in_=bcast(colbuf[s, 2], CAP))
                y1B = pb.tile([128, CAP], f32, tag="y1B")
                nc.gpsimd.dma_start(out=y1B[:], in_=bcast(colbuf[s, 3], CAP))
                y2B = pb.tile([128, CAP], f32, tag="y2B")
                nc.gpsimd.dma_start(out=y2B[:], in_=bcast(colbuf[s, 4], CAP))
                a3B = pb.tile([128, CAP], f32, tag="a3B")
                nc.gpsimd.dma_start(out=a3B[:], in_=bcast(colbuf[s, 5], CAP))

                VS = pw.tile([128, NT], f32, tag="VS")
                nc.vector.tensor_scalar(out=VS[:], in0=SRT[:, :, 7], scalar1=float(COND),
                                        scalar2=None, op0=A.is_gt)

                jleadB = pb.tile([128, CAP], f32, tag="jleadB")
                nc.vector.memset(jleadB[:], 0.0)

                # ---- F: blocked greedy scan
                CID = pw.tile([128, NT], f32, tag="CID")
                LEADC = pw.tile([128, NT], f32, tag="LEADC")

                def ov_block(x1i, x2i, y1i, y2i, a3i, c0, L, tagp):
                    # overlap bool [128, L] of candidates (per-part scalars) vs
                    # sorted rows c0:c0+L (broadcast rows)
                    mnx2 = pk.tile([128, L], f32, tag=tagp + "mnx2")
                    nc.vector.tensor_scalar(out=mnx2[:], in0=x2B[:, c0:c0 + L],
                                            scalar1=x2i, scalar2=None, op0=A.min)
                    ixn = pk.tile([128, L], f32, tag=tagp + "ixn")
                    nc.vector.scalar_tensor_tensor(out=ixn[:], in0=x1B[:, c0:c0 + L],
                                                   scalar=x1i, in1=mnx2[:],
                                                   op0=A.max, op1=A.subtract)
                    ixr = pk.tile([128, L], f32, tag=tagp + "ixr")
                    nc.vector.tensor_scalar(out=ixr[:], in0=ixn[:], scalar1=-1.0,
                                            scalar2=0.0, op0=A.mult, op1=A.max)
                    mny2 = pk.tile([128, L], f32, tag=tagp + "mny2")
                    nc.vector.tensor_scalar(out=mny2[:], in0=y2B[:, c0:c0 + L],
                                            scalar1=y2i, scalar2=None, op0=A.min)
                    iyn = pk.tile([128, L], f32, tag=tagp + "iyn")
                    nc.vector.scalar_tensor_tensor(out=iyn[:], in0=y1B[:, c0:c0 + L],
                                                   scalar=y1i, in1=mny2[:],
                                                   op0=A.max, op1=A.subtract)
                    iyr = pk.tile([128, L], f32, tag=tagp + "iyr")
                    nc.vector.tensor_scalar(out=iyr[:], in0=iyn[:], scalar1=-1.0,
                                            scalar2=0.0, op0=A.mult, op1=A.max)
                    inter = pk.tile([128, L], f32, tag=tagp + "inter")
                    nc.vector.tensor_tensor(out=inter[:], in0=ixr[:], in1=iyr[:], op=A.mult)
                    sp = pk.tile([128, L], f32, tag=tagp + "sp")
                    nc.vector.tensor_scalar(out=sp[:], in0=a3B[:, c0:c0 + L],
                                            scalar1=a3i, scalar2=None, op0=A.add)
                    uu = pk.tile([128, L], f32, tag=tagp + "uu")
                    nc.vector.scalar_tensor_tensor(out=uu[:], in0=inter[:], scalar=-float(IOU),
                                                   in1=sp[:], op0=A.mult, op1=A.add)
                    ovb = pk.tile([128, L], f32, tag=tagp + "ovb")
                    nc.vector.tensor_tensor(out=ovb[:], in0=inter[:], in1=uu[:], op=A.is_gt)
                    return ovb

                for b in range(NBLK):
                    x1i = SRT[:, b, 8:9]
                    x2i = SRT[:, b, 9:10]
                    y1i = SRT[:, b, 10:11]
                    y2i = SRT[:, b, 11:12]
                    a3i = SRT[:, b, 12:13]
                    runmin = pk.tile([128, 1], f32, tag="runmin")
                    nc.vector.memset(runmin[:], 0.0)
                    # ext suppression + cid-base over prefix blocks
                    for c0 in range(0, b * 128, 512):
                        L = min(512, b * 128 - c0)
                        ovb = ov_block(x1i, x2i, y1i, y2i, a3i, c0, L, "g")
                        sel = pk.tile([128, L], f32, tag="esel")
                        nc.vector.tensor_tensor(out=sel[:], in0=ovb[:],
                                                in1=jleadB[:, c0:c0 + L], op=A.mult)
                        cmn = pk.tile([128, 1], f32, tag="cmn")
                        nc.vector.tensor_reduce(out=cmn[:], in_=sel[:], axis=AX, op=A.min)
                        nc.vector.tensor_tensor(out=runmin[:], in0=runmin[:], in1=cmn[:],
                                                op=A.min)
                    supp = pk.tile([128, 1], f32, tag="supp")
                    nc.vector.tensor_scalar(out=supp[:], in0=runmin[:], scalar1=-8192.0,
                                            scalar2=None, op0=A.is_lt)
                    # within-block overlap matrix + strict-lower mask, transposed
                    ovblk = ov_block(x1i, x2i, y1i, y2i, a3i, b * 128, 128, "g")
                    Mb = pk.tile([128, 128], f32, tag="Mb")
                    nc.vector.tensor_tensor(out=Mb[:], in0=ovblk[:], in1=LT128[:], op=A.mult)
                    MbTp = ps.tile([128, 128], f32, tag="MbTp", space="PSUM")
                    nc.tensor.transpose(out=MbTp[:], in_=Mb[:], identity=ident[:])
                    MbT = pk.tile([128, 128], f32, tag="MbT")
                    nc.vector.tensor_copy(out=MbT[:], in_=MbTp[:])
                    # rounds
                    notsupp = pk.tile([128, 1], f32, tag="notsupp")
                    nc.vector.tensor_scalar(out=notsupp[:], in0=supp[:], scalar1=-1.0,
                                            scalar2=1.0, op0=A.mult, op1=A.add)
                    u = pk.tile([128, 1], f32, tag="u")
                    nc.vector.tensor_tensor(out=u[:], in0=VS[:, b:b + 1], in1=notsupp[:],
                                            op=A.mult)
                    lvec = pk.tile([128, 1], f32, tag="lvec")
                    nc.vector.memset(lvec[:], 0.0)
                    for r in range(RROUNDS):
                        p1 = ps1.tile([128, 1], f32, tag="p1", space="PSUM")
                        nc.tensor.matmul(out=p1[:], lhsT=MbT[:], rhs=u[:],
                                         start=True, stop=True)
                        e1 = pk.tile([128, 1], f32, tag="e1")
                        nc.vector.tensor_scalar(out=e1[:], in0=p1[:], scalar1=0.0,
                                                scalar2=None, op0=A.is_equal)
                        newl = pk.tile([128, 1], f32, tag="newl")
                        nc.vector.tensor_tensor(out=newl[:], in0=u[:], in1=e1[:], op=A.mult)
                        nc.vector.tensor_tensor(out=lvec[:], in0=lvec[:], in1=newl[:],
                                                op=A.max)
                        if r < RROUNDS - 1:
                            p2 = ps1.tile([128, 1], f32, tag="p2", space="PSUM")
                            nc.tensor.matmul(out=p2[:], lhsT=MbT[:], rhs=newl[:],
                                             start=True, stop=True)
                            nd = pk.tile([128, 1], f32, tag="nd")
                            nc.vector.tensor_scalar(out=nd[:], in0=p2[:], scalar1=0.0,
                                                    scalar2=None, op0=A.is_equal)
                            nn = pk.tile([128, 1], f32, tag="nn")
                            nc.vector.tensor_scalar(out=nn[:], in0=newl[:], scalar1=-1.0,
                                                    scalar2=1.0, op0=A.mult, op1=A.add)
                            nc.vector.tensor_tensor(out=u[:], in0=u[:], in1=nd[:], op=A.mult)
                            nc.vector.tensor_tensor(out=u[:], in0=u[:], in1=nn[:], op=A.mult)
                    nc.vector.tensor_copy(out=LEADC[:, b:b + 1], in_=lvec[:])
                    # jlead segment: lvec * (i - 16384), broadcast to all partitions
                    iidx = pk.tile([128, 1], f32, tag="iidx")
                    nc.vector.tensor_scalar(out=iidx[:], in0=IOC[:], scalar1=float(b * 128),
                                            scalar2=-float(S16K), op0=A.add, op1=A.add)
                    jlv = pk.tile([128, 1], f32, tag="jlv")
                    nc.vector.tensor_tensor(out=jlv[:], in0=lvec[:], in1=iidx[:], op=A.mult)
                    nc.gpsimd.dma_start(out=coljl[s, b].rearrange("(p a) -> p a", a=1), in_=jlv[:])
                    nc.gpsimd.dma_start(out=jleadB[:, b * 128:(b + 1) * 128],
                                        in_=bcast(coljl[s, b], 128))
                    # local cid: min over in-block leaders overlapping i
                    lsel = pk.tile([128, 128], f32, tag="lsel")
                    nc.vector.tensor_tensor(out=lsel[:], in0=ovblk[:],
                                            in1=jleadB[:, b * 128:(b + 1) * 128], op=A.mult)
                    lmn = pk.tile([128, 1], f32, tag="lmn")
                    nc.vector.tensor_reduce(out=lmn[:], in_=lsel[:], axis=AX, op=A.min)
                    cmin = pk.tile([128, 1], f32, tag="cmin")
                    nc.vector.tensor_tensor(out=cmin[:], in0=runmin[:], in1=lmn[:], op=A.min)
                    cidv = pk.tile([128, 1], f32, tag="cidv")
                    nc.vector.tensor_scalar(out=cidv[:], in0=cmin[:], scalar1=float(S16K),
                                            scalar2=None, op0=A.add)
                    okc = pk.tile([128, 1], f32, tag="okc")
                    nc.vector.tensor_scalar(out=okc[:], in0=cidv[:], scalar1=8192.0,
                                            scalar2=None, op0=A.is_lt)
                    nc.vector.tensor_tensor(out=okc[:], in0=okc[:], in1=VS[:, b:b + 1],
                                            op=A.mult)
                    ok8 = pk.tile([128, 1], i8, tag="ok8")
                    nc.vector.tensor_copy(out=ok8[:], in_=okc[:])
                    nc.vector.select(out=CID[:, b:b + 1], mask=ok8[:], on_true=cidv[:],
                                     on_false=MONE[:])

                # ---- G: cluster rank (count of earlier members of same cluster)
                nc.gpsimd.dma_start(out=colbuf[s, 6].rearrange("(t p) -> p t", p=128),
                                    in_=CID[:])
                cidB = pb.tile([128, CAP], f32, tag="cidB")
                nc.gpsimd.dma_start(out=cidB[:], in_=bcast(colbuf[s, 6], CAP))
                RK = pw.tile([128, NT], f32, tag="RK")
                for t in range(NT):
                    iidx = pk.tile([128, 1], f32, tag="iidx")
                    nc.vector.tensor_scalar(out=iidx[:], in0=IOC[:], scalar1=float(t * 128),
                                            scalar2=None, op0=A.add)
                    nch = (t * 128 + 128 + 511) // 512
                    acc = None
                    for c in range(nch):
                        c0 = c * 512
                        L = min(512, CAP - c0)
                        lt = pk.tile([128, L], f32, tag="lt")
                        nc.vector.tensor_scalar(out=lt[:], in0=JROW[:, c0:c0 + L],
                                                scalar1=iidx[:, :1], scalar2=None, op0=A.is_lt)
                        part = pk.tile([128, 1], f32, tag="part")
                        nc.vector.scalar_tensor_tensor(out=junk[:, 0:L], in0=cidB[:, c0:c0 + L],
                                                       scalar=CID[:, t:t + 1], in1=lt[:],
                                                       op0=A.is_equal, op1=A.mult,
                                                       accum_out=part[:])
                        if acc is None:
                            acc = pk.tile([128, 1], f32, tag="acc")
                            nc.vector.tensor_copy(out=acc[:], in_=part[:])
                        else:
                            nc.vector.tensor_tensor(out=acc[:], in0=acc[:], in1=part[:], op=A.add)
                    nc.vector.tensor_copy(out=RK[:, t:t + 1], in_=acc[:])

                # ---- H: cluster-table scatter
                DST = pw.tile([128, NT], f32, tag="DST")
                for t in range(NT):
                    m15 = pk.tile([128, 1], f32, tag="m15")
                    nc.vector.tensor_scalar(out=m15[:], in0=RK[:, t:t + 1], scalar1=15.0,
                                            scalar2=None, op0=A.min)
                    d0 = pk.tile([128, 1], f32, tag="d0")
                    nc.vector.scalar_tensor_tensor(out=d0[:], in0=CID[:, t:t + 1],
                                                   scalar=16.0, in1=m15[:],
                                                   op0=A.mult, op1=A.add)
                    c1 = pk.tile([128, 1], f32, tag="c1")
                    nc.vector.tensor_scalar(out=c1[:], in0=CID[:, t:t + 1], scalar1=0.0,
                                            scalar2=None, op0=A.is_ge)
                    c2 = pk.tile([128, 1], f32, tag="c2")
                    nc.vector.tensor_scalar(out=c2[:], in0=RK[:, t:t + 1], scalar1=16.0,
                                            scalar2=None, op0=A.is_lt)
                    nc.vector.tensor_tensor(out=c1[:], in0=c1[:], in1=c2[:], op=A.mult)
                    c18 = pk.tile([128, 1], i8, tag="c18")
                    nc.vector.tensor_copy(out=c18[:], in_=c1[:])
                    nc.vector.select(out=DST[:, t:t + 1], mask=c18[:], on_true=d0[:],
                                     on_false=DUMPC[:])
                DSTi = pw.tile([128, NT], i32, tag="DSTi")
                nc.vector.tensor_copy(out=DSTi[:], in_=DST[:])
                for t in range(NT):
                    nc.gpsimd.indirect_dma_start(
                        out=ctab[s][:], in_=SRT[:, t, 0:7],
                        out_offset=bass.IndirectOffsetOnAxis(ap=DSTi[:, t:t + 1], axis=0),
                        in_offset=None)

                # ---- I: MLP + softmax + merge
                CL = pw.tile([128, NT, 112], f32, tag="CL")
                nc.gpsimd.dma_start(
                    out=CL[:],
                    in_=ctab[s][0:NT * 128 * 16].rearrange(
                        "(t p sl) c -> p t (sl c)", p=128, sl=16))
                CLT = pb.tile([112, CAP], f32, tag="CLT")
                for t in range(NT):
                    tp = ps.tile([112, 128], f32, tag="MbTp", name="tp", space="PSUM")
                    nc.tensor.transpose(out=tp[:], in_=CL[:, t, :], identity=ident[:])
                    nc.vector.tensor_copy(out=CLT[:, t * 128:(t + 1) * 128], in_=tp[:])
                LG = pw.tile([128, NT, 64], f32, tag="LG")
                for cc in range(3):
                    c0 = cc * 512
                    hs = pk.tile([128, 512], f32, tag="hs")
                    logitsP = ps.tile([16, 512], f32, tag="logitsP", space="PSUM")
                    absP = ps.tile([16, 512], f32, tag="absP", space="PSUM")
                    for bb in range(8):
                        accp = ps1.tile([128, 512], f32, tag="accp", space="PSUM")
                        nc.tensor.matmul(out=accp[:], lhsT=W1T[bb][:],
                                         rhs=CLT[:, c0:c0 + 512], start=True, stop=True)
                        nc.scalar.activation(out=hs[:], in_=accp[:], func=AF.Relu,
                                             bias=B1C[:, :1], scale=1.0)
                        nc.tensor.matmul(out=logitsP[:], lhsT=W2T[bb][:], rhs=hs[:],
                                         start=(bb == 0), stop=(bb == 7))
                    clabs = pk.tile([112, 512], f32, tag="clabs")
                    nc.vector.scalar_tensor_tensor(out=clabs[:], in0=CLT[:, c0:c0 + 512],
                                                   scalar=-1.0, in1=CLT[:, c0:c0 + 512],
                                                   op0=A.mult, op1=A.max)
                    nc.tensor.matmul(out=absP[:], lhsT=ONESBD[:], rhs=clabs[:],
                                     start=True, stop=True)
                    LGS = pk.tile([128, 512], f32, tag="LGS")
                    nc.vector.tensor_scalar(out=LGS[0:16, :], in0=logitsP[:],
                                            scalar1=B2C[:, :1], scalar2=None, op0=A.add)
                    nc.vector.tensor_copy(out=LGS[32:48, :], in_=absP[:])
                    for q in range(4):
                        tp2 = ps.tile([128, 128], f32, tag="MbTp", name="tp2", space="PSUM")
                        nc.tensor.transpose(out=tp2[:], in_=LGS[:, q * 128:(q + 1) * 128],
                                            identity=ident[:])
                        nc.vector.tensor_copy(out=LG[:, cc * 4 + q, :], in_=tp2[:, 0:64])
                INF = pw.tile([128, NT, 9], f32, tag="INF")
                for t in range(NT):
                    lg = LG[:, t, 0:16]
                    ab = LG[:, t, 32:48]
                    okf = pk.tile([128, 16], f32, tag="okf")
                    nc.vector.tensor_scalar(out=okf[:], in0=ab[:], scalar1=0.0,
                                            scalar2=None, op0=A.is_gt)
                    okb = pk.tile([128, 16], i8, tag="okb")
                    nc.vector.tensor_copy(out=okb[:], in_=okf[:])
                    lm = pk.tile([128, 16], f32, tag="lm")
                    nc.vector.select(out=lm[:], mask=okb[:], on_true=lg[:], on_false=NEG9[:])
                    mx = pk.tile([128, 1], f32, tag="mx")
                    nc.vector.tensor_reduce(out=mx[:], in_=lm[:], axis=AX, op=A.max)
                    sh = pk.tile([128, 16], f32, tag="sh")
                    nc.vector.tensor_scalar(out=sh[:], in0=lm[:], scalar1=mx[:, :1],
                                            scalar2=None, op0=A.subtract)
                    ex = pk.tile([128, 16], f32, tag="ex")
                    nc.scalar.activation(out=ex[:], in_=sh[:], func=AF.Exp, bias=0.0, scale=1.0)
                    sm = pk.tile([128, 1], f32, tag="sm")
                    nc.vector.tensor_reduce(out=sm[:], in_=ex[:], axis=AX, op=A.add)
                    rcs = pk.tile([128, 1], f32, tag="rcs")
                    nc.vector.reciprocal(out=rcs[:], in_=sm[:])
                    wgt = pk.tile([128, 16], f32, tag="wgt")
                    nc.vector.tensor_scalar(out=wgt[:], in0=ex[:], scalar1=rcs[:, :1],
                                            scalar2=None, op0=A.mult)
                    WX = pk.tile([128, 112], f32, tag="WX")
                    wx3 = WX[:].rearrange("p (s d) -> p s d", d=7)
                    for d in range(7):
                        nc.vector.tensor_copy(out=wx3[:, :, d], in_=wgt[:])
                    wq = pk.tile([128, 112], f32, tag="wq")
                    nc.vector.tensor_tensor(out=wq[:], in0=CL[:, t, :], in1=WX[:], op=A.mult)
                    f56 = pk.tile([128, 56], f32, tag="f56")
                    nc.vector.tensor_tensor(out=f56[:], in0=wq[:, 0:56], in1=wq[:, 56:112],
                                            op=A.add)
                    f28 = pk.tile([128, 28], f32, tag="f28")
                    nc.vector.tensor_tensor(out=f28[:], in0=f56[:, 0:28], in1=f56[:, 28:56],
                                            op=A.add)
                    f14 = pk.tile([128, 14], f32, tag="f14")
                    nc.vector.tensor_tensor(out=f14[:], in0=f28[:, 0:14], in1=f28[:, 14:28],
                                            op=A.add)
                    mg = pk.tile([128, 7], f32, tag="mg")
                    nc.vector.tensor_tensor(out=mg[:], in0=f14[:, 0:7], in1=f14[:, 7:14],
                                            op=A.add)
                    # size fallback
                    sneg = pk.tile([128, 3], f32, tag="sneg")
                    nc.vector.tensor_scalar(out=sneg[:], in0=mg[:, 3:6], scalar1=0.0,
                                            scalar2=None, op0=A.is_le)
                    sn8 = pk.tile([128, 3], i8, tag="sn8")
                    nc.vector.tensor_copy(out=sn8[:], in_=sneg[:])
                    szf = pk.tile([128, 3], f32, tag="szf")
                    nc.vector.select(out=szf[:], mask=sn8[:], on_true=SRT[:, t, 3:6],
                                     on_false=mg[:, 3:6])
                    nc.vector.tensor_copy(out=INF[:, t, 0:3], in_=mg[:, 0:3])
                    nc.vector.tensor_copy(out=INF[:, t, 3:6], in_=szf[:])
                    nc.vector.tensor_copy(out=INF[:, t, 6:7], in_=mg[:, 6:7])
                    nc.vector.tensor_copy(out=INF[:, t, 7:8], in_=SRT[:, t, 13:14])
                    nc.vector.tensor_scalar(out=INF[:, t, 8:9], in0=CLABc[:], scalar1=0.0,
                                            scalar2=None, op0=A.add)
                    nc.vector.tensor_scalar(out=INF[:, t, :], in0=INF[:, t, :],
                                            scalar1=LEADC[:, t:t + 1], scalar2=None,
                                            op0=A.mult)

                # ---- J: scatter to global rows
                if dbgx is not None:
                    DBG = pw.tile([128, 32], f32, tag="DBG")
                    nc.vector.tensor_copy(out=DBG[:, 0:12], in_=SRT[:, :, 14])
                    nc.vector.tensor_copy(out=DBG[:, 12:24], in_=LEADC[:])
                    nc.vector.tensor_copy(out=DBG[:, 24:32], in_=INF[:, 0, 0:8])
                    nc.sync.dma_start(out=dbgx[s], in_=DBG[:])
                # invalid rows (key<=COND, incl. zero pads) -> OOB index so the
                # bounds check skips them instead of clobbering real rows
                vs8 = pw.tile([128, NT], i8, tag="vs8")
                nc.vector.tensor_copy(out=vs8[:], in_=VS[:])
                OOBC = pw.tile([128, NT], f32, tag="OOBC")
                nc.vector.memset(OOBC[:], float(N))
                GSf = pw.tile([128, NT], f32, tag="GSf")
                nc.vector.select(out=GSf[:], mask=vs8[:], on_true=SRT[:, :, 14],
                                 on_false=OOBC[:])
                GS = pw.tile([128, NT], i32, tag="GS")
                nc.vector.tensor_copy(out=GS[:], in_=GSf[:])
                for t in range(NT):
                    nc.gpsimd.indirect_dma_start(
                        out=oinfo[s][:], in_=INF[:, t, :],
                        out_offset=bass.IndirectOffsetOnAxis(ap=GS[:, t:t + 1], axis=0),
                        in_offset=None, bounds_check=bcreg, oob_is_err=False)
                    nc.gpsimd.indirect_dma_start(
                        out=olead[s][:].rearrange("(n a) -> n a", a=1),
                        in_=LEADC[:, t:t + 1],
                        out_offset=bass.IndirectOffsetOnAxis(ap=GS[:, t:t + 1], axis=0),
                        in_offset=None, bounds_check=bcreg, oob_is_err=False)

    return nc


def _prep_weights(W1, b1, W2, b2):
    w1bd = np.zeros((8, 112, 128), np.float32)
    w2bd = np.zeros((8, 128, 16), np.float32)
    for bb in range(8):
        for sp in range(2):
            s = 2 * bb + sp
            w1bd[bb, 7 * s:7 * s + 7, 64 * sp:64 * sp + 64] = W1
            w2bd[bb, 64 * sp:64 * sp + 64, s] = W2[:, 0]
    onesbd = np.zeros((112, 16), np.float32)
    for s in range(16):
        onesbd[7 * s:7 * s + 7, s] = 1.0
    b1c = np.concatenate([b1, b1]).astype(np.float32).reshape(128, 1)
    b2c = np.full((16, 1), np.float32(b2[0]), np.float32)
    return w1bd, w2bd, onesbd, b1c, b2c


def _make_runner(nc):
    """Build a persistent jitted 8-core executor (run_bass_via_pjrt re-jits
    on every call; we cache the jit so repeat calls only execute)."""
    import jax
    import numpy as np
    import concourse.mybir as mybir
    from concourse import bass2jax
    from jax.sharding import Mesh, PartitionSpec
    from jax.experimental.shard_map import shard_map

    bass2jax.install_neuronx_cc_hook()
    n_cores = 8
    partition_name = nc.partition_id_tensor.name if nc.partition_id_tensor else None
    in_names, out_names, out_avals, zero_shapes = [], [], [], []
    for alloc in nc.m.functions[0].allocations:
        if not isinstance(alloc, mybir.MemoryLocationSet):
            continue
        name = alloc.memorylocations[0].name
        if alloc.kind == "ExternalInput":
            if name != partition_name:
                in_names.append(name)
        elif alloc.kind == "ExternalOutput":
            out_names.append(name)
            shape = tuple(alloc.tensor_shape)
            dtype = mybir.dt.np(alloc.dtype)
            out_avals.append(jax.core.ShapedArray(shape, dtype))
            zero_shapes.append((shape, dtype))
    n_params = len(in_names)
    n_outs = len(out_avals)
    all_names = in_names + out_names + ([partition_name] if partition_name else [])
    donate = tuple(range(n_params, n_params + n_outs))

    def _body(*args):
        operands = list(args)
        if partition_name is not None:
            operands.append(bass2jax.partition_id_tensor())
        outs = bass2jax._bass_exec_p.bind(
            *operands,
            out_avals=tuple(out_avals),
            in_names=tuple(all_names),
            out_names=tuple(out_names),
            lowering_input_output_aliases=(),
            sim_require_finite=True,
            sim_require_nnan=True,
            nc=nc,
        )
        return tuple(outs)

    devices = jax.devices()[:n_cores]
    mesh = Mesh(np.asarray(devices), ("core",))
    in_specs = (PartitionSpec("core"),) * (n_params + n_outs)
    out_specs = (PartitionSpec("core"),) * n_outs
    sharded = jax.jit(shard_map(_body, mesh=mesh, in_specs=in_specs,
                                out_specs=out_specs, check_rep=False),
                      donate_argnums=donate, keep_unused=True)

    def run(in_maps):
        concat_in = [np.concatenate([np.asarray(m[nm]) for m in in_maps], axis=0)
                     for nm in in_names]
        concat_zeros = [np.zeros((n_cores * sh[0], *sh[1:]), dt)
                        for sh, dt in zero_shapes]
        out_arrs = sharded(*concat_in, *concat_zeros)
        return [{nm: np.asarray(out_arrs[i]).reshape(n_cores, *out_avals[i].shape)[c]
                 for i, nm in enumerate(out_names)} for c in range(n_cores)]

    return run


def kernel(boxes, scores, labels, W1, b1, W2, b2):

    boxes = np.asarray(boxes, np.float32)
    scores = np.asarray(scores, np.float32)
    labels = np.asarray(labels)
    w1bd, w2bd, onesbd, b1c, b2c = _prep_weights(
        np.asarray(W1, np.float32), np.asarray(b1, np.float32),
        np.asarray(W2, np.float32), np.asarray(b2, np.float32))

    # shard (frame, class) -> (core, slot)
    shards = [(f, c) for f in range(B) for c in range(3)]
    place = {}
    for k in range(8):
        place[(k, 0)] = shards[k]
        if 8 + k < len(shards):
            place[(k, 1)] = shards[8 + k]

    in_maps = []
    for k in range(8):
        m = {"w1bd": w1bd, "w2bd": w2bd, "onesbd": onesbd, "b1c": b1c, "b2c": b2c}
        cb = np.zeros((2, CAP, 7), np.float32)
        cs = np.zeros((2, CAP), np.float32)
        co = np.full((2, CAP), PADORIG, np.float32)
        fs = np.zeros((2, N), np.float32)
        cl = np.zeros((2, 128), np.float32)
        for slot in range(2):
            if (k, slot) in place:
                f, c = place[(k, slot)]
                sel = np.where(labels[f] == c)[0]
                ncl = len(sel)
                assert ncl <= CAP, f"class count {ncl} exceeds CAP {CAP}"
                cb[slot, :ncl] = boxes[f, sel]
                cs[slot, :ncl] = scores[f, sel]
                co[slot, :ncl] = sel.astype(np.float32)
                fs[slot] = scores[f]
                cl[slot, :] = np.float32(c)
        m.update(cboxes=cb, cscore=cs, corig=co, fscore=fs, clab=cl)
        in_maps.append(m)

    run = _cache.get("run")
    if run is None:
        nc = _build_program()
        run = _make_runner(nc)
        _cache["run"] = run

    results = run(in_maps)

    info = np.zeros((B, N, 9), np.float32)
    lead = np.zeros((B, N), bool)
    for (k, slot), (f, c) in place.items():
        info[f] += results[k][f"oinfo{slot}"]
        lead[f] |= results[k][f"olead{slot}"] > 0.5
    return info, lead
